# revision 1
# baseline (speedup 1.0000x reference)
"""Multi-head attention Trainium2 kernel (8 NeuronCores, SPMD).

Problem: B=4, T=2048, n_feat=512, H=8 heads, d_k=64.
Sharding: core c -> batch b = c//2, head-half hh = c%2 (4 heads = 256 attn dims).
Each core computes, for its (b, head-half):
    Q^T/K^T projections in [o, t] layout, V in [t, o] layout (+ ones column),
    flash-style attention with scores transposed (S^T[j, i]) so the softmax
    denominator comes out of the PV matmul for free, then the partial output
    projection out^T = Wo_blk @ x^T in [o2, t] layout.
Host sums the two head-half partials per batch, transposes, adds bo.

Matmuls run in float32r (TF32-like, ~1e-4 rel err, full PE rate); exp on ACT.
"""
import sys

sys.path.insert(0, "/opt/trn_rl_repo")

import numpy as np

import concourse.bass as bass
import concourse.tile as tile
from concourse import bacc, mybir
from concourse.bass_utils import run_bass_kernel_spmd

P = 128
T = 2048
F = 512            # n_feat (projection contraction dim)
OB = 256           # per-core attention dims (4 heads x 64)
NH = 4             # local heads
DK = 64
NT = T // P        # 16 row tiles
FO = F // P        # 4 feature tiles
NSUP = 2           # i-supers per head
ISUP = T // NSUP   # 1024
NC_ = ISUP // P    # 8 chunks per super
JT = NT            # 16 j tiles
NEG = -1.0e30
EPS = 1e-8

f32 = mybir.dt.float32
f32r = mybir.dt.float32r

_CACHE = {}


def _build():
    nc = bacc.Bacc("TRN2", target_bir_lowering=False, debug=False, num_devices=8)

    xq = nc.dram_tensor("xq", (T, F), f32, kind="ExternalInput").ap()
    xk = nc.dram_tensor("xk", (T, F), f32, kind="ExternalInput").ap()
    xv = nc.dram_tensor("xv", (T, F), f32, kind="ExternalInput").ap()
    wq = nc.dram_tensor("wq", (OB, F), f32, kind="ExternalInput").ap()
    wk = nc.dram_tensor("wk", (OB, F), f32, kind="ExternalInput").ap()
    wv = nc.dram_tensor("wv", (OB, F), f32, kind="ExternalInput").ap()
    wo = nc.dram_tensor("wo", (F, OB), f32, kind="ExternalInput").ap()
    bqr = nc.dram_tensor("bqr", (P, OB // P), f32, kind="ExternalInput").ap()
    bkr = nc.dram_tensor("bkr", (P, OB // P), f32, kind="ExternalInput").ap()
    bvb = nc.dram_tensor("bvb", (P, OB), f32, kind="ExternalInput").ap()
    mb = nc.dram_tensor("mb", (P, JT), f32, kind="ExternalInput").ap()
    ident = nc.dram_tensor("ident", (P, P), f32, kind="ExternalInput").ap()
    outT = nc.dram_tensor("outT", (F, T), f32, kind="ExternalOutput").ap()

    with tile.TileContext(nc) as tc:
        with tc.tile_pool(name="const", bufs=1) as cpool, \
             tc.tile_pool(name="persist", bufs=1) as ppool, \
             tc.tile_pool(name="win", bufs=2) as wpool, \
             tc.tile_pool(name="inp", bufs=1) as ipool, \
             tc.tile_pool(name="stage", bufs=4) as spool, \
             tc.tile_pool(name="et", bufs=3) as epool, \
             tc.tile_pool(name="norm", bufs=2) as npool, \
             tc.tile_pool(name="ps", bufs=2, space="PSUM") as ps:

            def big_ps(name):
                # "big" tag: 2-bank slots shared by S^T / proj / dance tiles
                return ps.tile([P, ISUP], f32, tag="big", name=name)

            def xp_ps(name):
                # "xp" tag: 2-bank slots shared by PV accum / transpose staging
                return ps.tile([P, ISUP], f32, tag="xp", name=name)

            # ---- constants ----
            id_sb = cpool.tile([P, P], f32, tag="ident")
            nc.sync.dma_start(out=id_sb[:], in_=ident[:])
            bq_sb = cpool.tile([P, OB // P], f32, tag="bq")
            nc.gpsimd.dma_start(out=bq_sb[:], in_=bqr[:])
            bk_sb = cpool.tile([P, OB // P], f32, tag="bk")
            nc.gpsimd.dma_start(out=bk_sb[:], in_=bkr[:])
            bv_sb = cpool.tile([P, OB], f32, tag="bv")
            nc.gpsimd.dma_start(out=bv_sb[:], in_=bvb[:])
            mb_sb = cpool.tile([P, JT], f32, tag="mb")
            nc.gpsimd.dma_start(out=mb_sb[:], in_=mb[:])

            # ---- weight transpose helpers (emitted per-tensor below) ----
            wT = {}

            def emit_wT(name, wdram):
                w_sb = wpool.tile([P, OB // P, F], f32, tag="wstage")
                nc.gpsimd.dma_start(
                    out=w_sb[:], in_=wdram.rearrange("(po p) f -> p po f", p=P)
                )
                wt = cpool.tile([P, FO, OB], f32r, tag=f"w{name}T")
                for fo in range(FO):
                    tp = xp_ps(f"wtr_{name}_{fo}")
                    for po in range(OB // P):
                        nc.tensor.transpose(
                            tp[:, po * P:(po + 1) * P],
                            w_sb[:, po, fo * P:(fo + 1) * P],
                            id_sb[:],
                        )
                    nc.scalar.copy(wt[:, fo, :], tp[:, :OB])
                wT[name] = wt

            def emit_woT():
                wo_sb = wpool.tile([P, FO, OB], f32, tag="wstage")
                nc.sync.dma_start(
                    out=wo_sb[:], in_=wo.rearrange("(a p) o -> p a o", p=P)
                )
                woT = cpool.tile([DK, NH, F], f32r, tag="woT")
                for h in range(NH):
                    tp = xp_ps(f"wotr_{h}")
                    for a in range(FO):
                        nc.tensor.transpose(
                            tp[:DK, a * P:(a + 1) * P],
                            wo_sb[:, a, h * DK:(h + 1) * DK],
                            id_sb[:],
                        )
                    nc.scalar.copy(woT[:, h, :], tp[:DK, :F])
                return woT

            # ---- persistent activations ----
            QT = ppool.tile([P, OB // P, T], f32r, tag="QT")
            KT = ppool.tile([P, OB // P, T], f32r, tag="KT")
            xT = ppool.tile([DK, NH, T], f32r, tag="xT")
            V2 = ppool.tile([P, NT, NH, DK + 1], f32r, tag="V2")
            one_sb = cpool.tile([P, NT * NH], f32, tag="ones")
            nc.vector.memset(one_sb[:], 1.0)
            nc.vector.tensor_copy(
                V2[:, :, :, DK:DK + 1],
                one_sb[:].rearrange("p (t h) -> p t h ()", t=NT),
            )

            # ---- phase 1: input transpose + projections ----
            def load_transposed(xdram, name):
                """x [T, F] -> inT [P, FO, T] f32r (partition = f%128)."""
                inT = ipool.tile([P, FO, T], f32r, tag="inT")
                xr = xdram.rearrange("(t p) f -> p t f", p=P)
                for g in range(NT // 2):
                    xs = spool.tile([P, 2, F], f32, tag="xs")
                    dma_eng = nc.sync if g % 2 == 0 else nc.gpsimd
                    dma_eng.dma_start(out=xs[:], in_=xr[:, 2 * g:2 * (g + 1), :])
                    t = 2 * g
                    tp = xp_ps(f"itr_{name}_{t}")
                    for i in range(2):
                        for fo in range(FO):
                            nc.tensor.transpose(
                                tp[:, i * F + fo * P:i * F + (fo + 1) * P],
                                xs[:, i, fo * P:(fo + 1) * P],
                                id_sb[:],
                            )
                    src_ap = tp[:, :2 * F].rearrange(
                        "p (i fo q) -> p fo i q", i=2, fo=FO
                    )
                    dst_ap = inT[:, :, t * P:(t + 2) * P].rearrange(
                        "p fo (i q) -> p fo i q", i=2
                    )
                    if g % 2 == 0:
                        nc.scalar.copy(dst_ap, src_ap)
                    else:
                        nc.vector.tensor_copy(dst_ap, src_ap)
                return inT

            def emit_qk_proj(name, bias_sb, dst, inT, po):
                for c in range(T // F):
                    pp = big_ps(f"proj_{name}_{po}_{c}")
                    for fo in range(FO):
                        nc.tensor.matmul(
                            pp[:, :F],
                            wT[name][:, fo, po * P:(po + 1) * P],
                            inT[:, fo, c * F:(c + 1) * F],
                            start=(fo == 0),
                            stop=(fo == FO - 1),
                        )
                    nc.vector.tensor_scalar_add(
                        dst[:, po, c * F:(c + 1) * F],
                        pp[:, :F],
                        bias_sb[:, po:po + 1],
                    )

            # K first (scores need all of KT po=0), then Q, then V
            emit_wT("k", wk)
            inT_k = load_transposed(xk, "k")
            emit_qk_proj("k", bk_sb, KT, inT_k, 0)
            emit_qk_proj("k", bk_sb, KT, inT_k, 1)
            emit_wT("q", wq)
            inT_q = load_transposed(xq, "q")
            emit_qk_proj("q", bq_sb, QT, inT_q, 0)
            emit_qk_proj("q", bq_sb, QT, inT_q, 1)

            # V: natural [t, o] layout + bias, interleaved into V2
            emit_wT("v", wv)
            inT_v = load_transposed(xv, "v")
            for t in range(NT):
                pp = big_ps(f"proj_v_{t}")
                for fo in range(FO):
                    nc.tensor.matmul(
                        pp[:, :OB],
                        inT_v[:, fo, t * P:(t + 1) * P],
                        wT["v"][:, fo, :],
                        start=(fo == 0),
                        stop=(fo == FO - 1),
                    )
                nc.vector.tensor_add(
                    V2[:, t, :, 0:DK],
                    pp[:, :OB].rearrange("p (h d) -> p h d", h=NH),
                    bv_sb[:].rearrange("p (h d) -> p h d", h=NH),
                )
            # ---- phase 2: attention ----
            def emit_jloop(h, su, dance_cb=None, jt_cb=None):
                qoff = (h % 2) * DK
                qpo = h // 2
                isl = su * ISUP
                xp = xp_ps(f"xp_{h}_{su}")

                def scores(jt):
                    st = big_ps(f"st_{h}_{su}_{jt}")
                    for c in range(ISUP // F):
                        nc.tensor.matmul(
                            st[:, c * F:(c + 1) * F],
                            KT[qoff:qoff + DK, qpo, jt * P:(jt + 1) * P],
                            QT[qoff:qoff + DK, qpo, isl + c * F:isl + (c + 1) * F],
                            start=True,
                            stop=True,
                        )
                    return st

                st_prev = scores(0)
                for jt in range(JT):
                    et = epool.tile([P, ISUP], f32r, tag="et")
                    nc.scalar.activation(
                        et[:],
                        st_prev[:],
                        mybir.ActivationFunctionType.Exp,
                        bias=mb_sb[:, jt:jt + 1],
                        scale=0.125,
                    )
                    if jt + 1 < JT:
                        st_prev = scores(jt + 1)
                    for c in range(ISUP // F):
                        nc.tensor.matmul(
                            xp[:DK + 1, c * F:(c + 1) * F],
                            V2[:, jt, h, :],
                            et[:, c * F:(c + 1) * F],
                            start=(jt == 0),
                            stop=(jt == JT - 1),
                        )
                    if jt == 2 and dance_cb is not None:
                        dance_cb()
                    if jt_cb is not None:
                        jt_cb(jt)
                return xp

            def emit_norm(h, su, xp):
                isl = su * ISUP
                # Z row (partition DK of xp psum) -> SBUF
                zst = npool.tile([1, ISUP], f32, tag="zrow")
                nc.vector.tensor_copy(zst[:], xp[DK:DK + 1, :ISUP])
                # transpose Z chunks onto partitions: zcol [P, NC_]
                zcol = big_ps(f"zcol_{h}_{su}")
                for c in range(NC_):
                    nc.tensor.transpose(
                        zcol[:, c:c + 1],
                        zst[:, c * P:(c + 1) * P],
                        id_sb[0:1, 0:1],
                    )
                # r = 1 / (Z + eps), partition-parallel
                zeps = npool.tile([P, NC_], f32, tag="zeps")
                nc.vector.tensor_scalar_add(zeps[:], zcol[:, :NC_], EPS)
                rcol = npool.tile([P, NC_], f32, tag="rcol")
                nc.vector.reciprocal(rcol[:], zeps[:])
                # transpose back into spare columns of the same psum tile:
                # rT rows [NC_, P] at cols [P, 2P) (disjoint from zcol's cols)
                rT_ps = zcol[:NC_, P:2 * P]
                nc.tensor.transpose(rT_ps, rcol[:], id_sb[:])
                rT_sb = npool.tile([NC_, P], f32, tag="rT_sb")
                nc.vector.tensor_copy(rT_sb[:], rT_ps)
                # gather rows into one [1, ISUP] SBUF row (partition shift via DMA)
                rrow = npool.tile([1, ISUP], f32, tag="zrow")
                for c in range(NC_):
                    (nc.sync if c % 2 == 0 else nc.gpsimd).dma_start(
                        out=rrow[:, c * P:(c + 1) * P],
                        in_=rT_sb[c:c + 1, :P],
                    )
                # broadcast across DK partitions (gpsimd)
                rb = npool.tile([DK, ISUP], f32, tag="rb")
                nc.gpsimd.partition_broadcast(rb[:], rrow[:])
                # x^T = x'^T * r
                nc.vector.tensor_mul(
                    xT[:, h, isl:isl + ISUP],
                    xp[0:DK, :ISUP],
                    rb[:],
                )

            woT = emit_woT()

            def emit_outproj(m2, half, psf):
                os2 = spool.tile([P, 2, F], f32, tag="os2")
                for cc in range(2):
                    c = 2 * half + cc
                    pp = psf(f"op_{m2}_{c}")
                    for h in range(NH):
                        nc.tensor.matmul(
                            pp[:, :F],
                            woT[:, h, m2 * P:(m2 + 1) * P],
                            xT[:, h, c * F:(c + 1) * F],
                            start=(h == 0),
                            stop=(h == NH - 1),
                        )
                    nc.vector.tensor_copy(os2[:, cc, :], pp[:, :F])
                nc.sync.dma_start(
                    out=outT[m2 * P:(m2 + 1) * P, half * 2 * F:(half + 1) * 2 * F],
                    in_=os2[:].rearrange("p c f -> p (c f)"),
                )

            pairs = [(h, su) for h in range(NH) for su in range(NSUP)]
            pending = [None]

            def dance_cb():
                if pending[0] is not None:
                    emit_norm(*pending[0])
                    pending[0] = None

            for idx, (h, su) in enumerate(pairs):
                if idx + 1 == len(pairs):
                    def late_cb(jt):
                        if jt == 3:
                            dance_cb()
                        elif jt in (6, 8, 10, 12):
                            emit_outproj((jt - 6) // 2, 0, xp_ps)
                    xp = emit_jloop(h, su, None, late_cb)
                else:
                    xp = emit_jloop(h, su, dance_cb)
                pending[0] = (h, su, xp)
            emit_norm(*pending[0])

            # ---- phase 3: remaining output projection (columns su=1) ----
            for m2 in range(F // P):
                emit_outproj(m2, 1, big_ps)

    nc.compile()
    return nc


def _prep_in_maps(query, key, value, mask, Wq, bq, Wk, bk, Wv, bv, Wo):
    ident = np.eye(P, dtype=np.float32)
    in_maps = []
    for c in range(8):
        b = c // 2
        hh = c % 2
        ob = slice(hh * OB, (hh + 1) * OB)
        mbias = np.where(mask[b, 0, :] == 0, np.float32(NEG), np.float32(0.0))
        mbias = np.ascontiguousarray(mbias.reshape(JT, P).T)
        in_maps.append({
            "xq": np.ascontiguousarray(query[b]),
            "xk": np.ascontiguousarray(key[b]),
            "xv": np.ascontiguousarray(value[b]),
            "wq": np.ascontiguousarray(Wq[ob, :]),
            "wk": np.ascontiguousarray(Wk[ob, :]),
            "wv": np.ascontiguousarray(Wv[ob, :]),
            "wo": np.ascontiguousarray(Wo[:, ob]),
            "bqr": np.ascontiguousarray(bq[ob].reshape(OB // P, P).T),
            "bkr": np.ascontiguousarray(bk[ob].reshape(OB // P, P).T),
            "bvb": np.ascontiguousarray(np.tile(bv[ob][None, :], (P, 1))),
            "mb": mbias,
            "ident": ident,
        })
    return in_maps


def kernel(query, key, value, mask, Wq, bq, Wk, bk, Wv, bv, Wo, bo):
    query = np.asarray(query, dtype=np.float32)
    key = np.asarray(key, dtype=np.float32)
    value = np.asarray(value, dtype=np.float32)
    mask = np.asarray(mask)
    Wq = np.asarray(Wq, dtype=np.float32)
    bq = np.asarray(bq, dtype=np.float32)
    Wk = np.asarray(Wk, dtype=np.float32)
    bk = np.asarray(bk, dtype=np.float32)
    Wv = np.asarray(Wv, dtype=np.float32)
    bv = np.asarray(bv, dtype=np.float32)
    Wo = np.asarray(Wo, dtype=np.float32)
    bo = np.asarray(bo, dtype=np.float32)

    if "nc" not in _CACHE:
        _CACHE["nc"] = _build()
    nc = _CACHE["nc"]

    B = query.shape[0]
    in_maps = _prep_in_maps(query, key, value, mask, Wq, bq, Wk, bk, Wv, bv, Wo)
    res = run_bass_kernel_spmd(nc, in_maps, core_ids=list(range(8)))

    out = np.empty((B, T, F), dtype=np.float32)
    for b in range(B):
        acc = res.results[2 * b]["outT"] + res.results[2 * b + 1]["outT"]
        out[b] = acc.T + bo[None, :]
    return out



# revision 5
# speedup vs baseline: 1.3810x; 1.3810x over previous
"""Multi-head attention Trainium2 kernel (8 NeuronCores, SPMD), v2.

Problem: B=4, T=2048, n_feat=512, H=8 heads, d_k=64.
Sharding: core c -> batch b = c//2, head-half hh = c%2 (4 heads = 256 attn dims).

Per-core dataflow (v2):
  - Host passes x^T [F, T] and W^T [F, OB] in bf16, Wo^T [OB, F] in f32 ->
    no on-device transposes for the projection phase.
  - Q^T/K^T [o, t] via W^T-stationary matmuls; V [t, o] natural + ones column
    (bf16) for the softmax denominator.
  - Attention: scores transposed S^T[j, i] (f32r, N=512); exp on ACT into
    bf16 E^T tiles; PV flipped: lhsT = E^T chunk, rhs = V||1 (bf16, N=65)
    accumulating x[i, o] + Z[i] in PSUM -> normalization is partition-parallel
    (reciprocal + broadcast multiply on DVE only), inline at loop end.
  - x blocks transposed on PE (f32r identity), out = x @ Wo^T in natural
    [T, F] layout; host sums the two head-half partials per batch and adds bo.
"""
import sys

sys.path.insert(0, "/opt/trn_rl_repo")

import numpy as np
import ml_dtypes

import concourse.bass as bass
import concourse.tile as tile
from concourse import bacc, mybir
from concourse.bass import broadcast_tensor_aps
from concourse.bass_utils import run_bass_kernel_spmd

P = 128
T = 2048
F = 512            # n_feat (projection contraction dim)
OB = 256           # per-core attention dims (4 heads x 64)
NH = 4             # local heads
DK = 64
NT = T // P        # 16 row tiles
FO = F // P        # 4 feature tiles
NSUP = 2           # i-supers
ISUP = T // NSUP   # 1024
JT = NT            # 16 j tiles
NEG = -1.0e30
EPS = 1e-8
DW = DK + 1        # PV output width per i-tile (x + Z)

f32 = mybir.dt.float32
f32r = mybir.dt.float32r
bf16 = mybir.dt.bfloat16

_CACHE = {}


def _build():
    nc = bacc.Bacc("TRN2", target_bir_lowering=False, debug=False, num_devices=8)

    xqT = nc.dram_tensor("xqT", (F, T), bf16, kind="ExternalInput").ap()
    xkT = nc.dram_tensor("xkT", (F, T), bf16, kind="ExternalInput").ap()
    xvT = nc.dram_tensor("xvT", (F, T), bf16, kind="ExternalInput").ap()
    wqT = nc.dram_tensor("wqT", (F, OB), bf16, kind="ExternalInput").ap()
    wkT = nc.dram_tensor("wkT", (F, OB), bf16, kind="ExternalInput").ap()
    wvT = nc.dram_tensor("wvT", (F, OB), bf16, kind="ExternalInput").ap()
    woT = nc.dram_tensor("woT", (OB, F), f32r, kind="ExternalInput").ap()
    bqr = nc.dram_tensor("bqr", (P, OB // P), f32, kind="ExternalInput").ap()
    bkr = nc.dram_tensor("bkr", (P, OB // P), f32, kind="ExternalInput").ap()
    bvb = nc.dram_tensor("bvb", (P, OB), f32, kind="ExternalInput").ap()
    mb = nc.dram_tensor("mb", (P, JT), f32, kind="ExternalInput").ap()
    ident = nc.dram_tensor("ident", (P, P), f32r, kind="ExternalInput").ap()
    outD = nc.dram_tensor("out", (T, F), f32, kind="ExternalOutput").ap()

    with tile.TileContext(nc) as tc:
        with tc.tile_pool(name="const", bufs=1) as cpool, \
             tc.tile_pool(name="xin", bufs=1) as xpool, \
             tc.tile_pool(name="persist", bufs=1) as ppool, \
             tc.tile_pool(name="et", bufs=3) as epool, \
             tc.tile_pool(name="norm", bufs=2) as npool, \
             tc.tile_pool(name="ost", bufs=2) as opool, \
             tc.tile_pool(name="ps", bufs=2, space="PSUM") as ps, \
             tc.tile_pool(name="psacc", bufs=1, space="PSUM") as psa:

            # ---- constant / weight loads (SP queue) ----
            wq_sb = cpool.tile([P, FO, OB], bf16, tag="wq")
            wk_sb = cpool.tile([P, FO, OB], bf16, tag="wk")
            wv_sb = cpool.tile([P, FO, OB], bf16, tag="wv")
            nc.sync.dma_start(out=wk_sb[:], in_=wkT.rearrange("(fo p) o -> p fo o", p=P))
            nc.sync.dma_start(out=wq_sb[:], in_=wqT.rearrange("(fo p) o -> p fo o", p=P))

            xq_sb = xpool.tile([P, FO, T], bf16, tag="xq")
            xk_sb = xpool.tile([P, FO, T], bf16, tag="xk")
            xv_sb = xpool.tile([P, FO, T], bf16, tag="xv")
            xkr = xkT.rearrange("(fo p) t -> p fo t", p=P)
            xqr = xqT.rearrange("(fo p) t -> p fo t", p=P)
            xvr = xvT.rearrange("(fo p) t -> p fo t", p=P)

            def xdma(sb, dr, c):
                nc.sync.dma_start(
                    out=sb[:, :, c * F:(c + 1) * F], in_=dr[:, :, c * F:(c + 1) * F]
                )

            xdma(xk_sb, xkr, 0)
            xdma(xq_sb, xqr, 0)
            xdma(xv_sb, xvr, 0)
            xdma(xq_sb, xqr, 1)

            bq_sb = cpool.tile([P, OB // P], f32, tag="bq")
            nc.sync.dma_start(out=bq_sb[:], in_=bqr[:])
            bk_sb = cpool.tile([P, OB // P], f32, tag="bk")
            nc.sync.dma_start(out=bk_sb[:], in_=bkr[:])
            bv_sb = cpool.tile([P, OB], f32, tag="bv")
            nc.sync.dma_start(out=bv_sb[:], in_=bvb[:])
            mb_sb = cpool.tile([P, JT], f32, tag="mb")
            nc.sync.dma_start(out=mb_sb[:], in_=mb[:])
            id_sb = cpool.tile([P, P], f32r, tag="ident")
            nc.sync.dma_start(out=id_sb[:], in_=ident[:])
            nc.sync.dma_start(out=wv_sb[:], in_=wvT.rearrange("(fo p) o -> p fo o", p=P))

            xdma(xk_sb, xkr, 1)
            xdma(xv_sb, xvr, 1)
            xdma(xk_sb, xkr, 2)
            xdma(xv_sb, xvr, 2)
            xdma(xk_sb, xkr, 3)
            xdma(xv_sb, xvr, 3)
            xdma(xq_sb, xqr, 2)
            xdma(xq_sb, xqr, 3)

            wo_sb = cpool.tile([P, 2, F], f32r, tag="wo")
            nc.sync.dma_start(out=wo_sb[:], in_=woT.rearrange("(pc p) f -> p pc f", p=P))

            # ---- persistent activations ----
            QT = ppool.tile([P, OB // P, T], f32r, tag="QT")
            KT = ppool.tile([P, OB // P, T], f32r, tag="KT")
            V2 = ppool.tile([P, NT, NH, DW], bf16, tag="V2")
            xnorm = ppool.tile([P, NT, OB], f32r, tag="xnorm")
            xT2 = ppool.tile([P, 2, T], f32r, tag="xT2")
            nc.vector.memset(V2[:, :, :, DK:DW], 1.0)

            # ---- projection pieces (interleaved into attention via cbs) ----
            def qk_proj(w_sb, x_sb, bias_sb, dst, otile, c):
                pp = ps.tile([P, F], f32, tag="pp")
                for fo in range(FO):
                    nc.tensor.matmul(
                        pp[:],
                        w_sb[:, fo, otile * P:(otile + 1) * P],
                        x_sb[:, fo, c * F:(c + 1) * F],
                        start=(fo == 0),
                        stop=(fo == FO - 1),
                    )
                nc.vector.tensor_scalar_add(
                    dst[:, otile, c * F:(c + 1) * F],
                    pp[:],
                    bias_sb[:, otile:otile + 1],
                )

            def v_proj(t):
                pp = ps.tile([P, F], f32, tag="pp")
                for fo in range(FO):
                    nc.tensor.matmul(
                        pp[:, :OB],
                        xv_sb[:, fo, t * P:(t + 1) * P],
                        wv_sb[:, fo, :],
                        start=(fo == 0),
                        stop=(fo == FO - 1),
                    )
                nc.vector.tensor_add(
                    V2[:, t, :, 0:DK],
                    pp[:, :OB].rearrange("p (h d) -> p h d", h=NH),
                    bv_sb[:].rearrange("p (h d) -> p h d", h=NH),
                )

            # ---- normalization halves (DVE only; inline at loop end) ----
            def norm_half(su, h, acc, g):
                accv = acc[:, g, 0:4 * DW].rearrange("p (r c) -> p r c", c=DW)
                zz = npool.tile([P, 4], f32, tag="zz")
                nc.vector.tensor_scalar_add(zz[:], accv[:, :, DK], EPS)
                rr = npool.tile([P, 4], f32, tag="rr")
                nc.vector.reciprocal(rr[:], zz[:])
                dst = xnorm[:, su * 8 + g * 4:su * 8 + (g + 1) * 4,
                            h * DK:(h + 1) * DK]
                r_ap = rr[:].rearrange("p a -> p a ()")
                r_b, a_b = broadcast_tensor_aps(r_ap, accv[:, :, 0:DK])
                nc.vector.tensor_mul(dst, a_b, r_b)

            # ---- attention j-loop ----
            def emit_jloop(su, h, cbs):
                qoff = (h % 2) * DK
                qpo = h // 2
                isl = su * ISUP
                acc = psa.tile([P, 2, F], f32, tag="acc")

                def scores(jt):
                    st = ps.tile([P, ISUP], f32, tag="big")
                    for cc in range(ISUP // F):
                        nc.tensor.matmul(
                            st[:, cc * F:(cc + 1) * F],
                            KT[qoff:qoff + DK, qpo, jt * P:(jt + 1) * P],
                            QT[qoff:qoff + DK, qpo, isl + cc * F:isl + (cc + 1) * F],
                            start=True,
                            stop=True,
                        )
                    return st

                st_prev = scores(0)
                for jt in range(JT):
                    et = epool.tile([P, ISUP], bf16, tag="et")
                    nc.scalar.activation(
                        et[:],
                        st_prev[:],
                        mybir.ActivationFunctionType.Exp,
                        bias=mb_sb[:, jt:jt + 1],
                        scale=0.125,
                    )
                    if jt + 1 < JT:
                        st_prev = scores(jt + 1)
                    for it in range(8):
                        # PSUM start zeroes the whole 2KB bank: only the first
                        # packed region per bank starts, only the last stops.
                        nc.tensor.matmul(
                            acc[:, it // 4, (it % 4) * DW:(it % 4) * DW + DW],
                            et[:, it * P:(it + 1) * P],
                            V2[:, jt, h, :],
                            start=(jt == 0 and it % 4 == 0),
                            stop=(jt == JT - 1 and it % 4 == 3),
                        )
                        if jt == JT - 1 and it == 3:
                            norm_half(su, h, acc, 0)
                    for cb in cbs.get(jt, ()):
                        cb()
                norm_half(su, h, acc, 1)

            # ---- x transpose + output projection pieces ----
            def transp(su, pc, half):
                tp = ps.tile([P, F], f32r, tag="pp")
                for k in range(4):
                    t = su * 8 + half * 4 + k
                    nc.tensor.transpose(
                        tp[:, k * P:(k + 1) * P],
                        xnorm[:, t, pc * P:(pc + 1) * P],
                        id_sb[:],
                    )
                nc.vector.tensor_copy(
                    xT2[:, pc, (su * 8 + half * 4) * P:(su * 8 + half * 4) * P + F],
                    tp[:],
                )

            def outproj(t):
                pp = ps.tile([P, F], f32, tag="pp")
                for pc in range(2):
                    nc.tensor.matmul(
                        pp[:],
                        xT2[:, pc, t * P:(t + 1) * P],
                        wo_sb[:, pc, :],
                        start=(pc == 0),
                        stop=(pc == 1),
                    )
                os = opool.tile([P, F], f32, tag="os")
                nc.vector.tensor_copy(os[:], pp[:])
                nc.sync.dma_start(out=outD[t * P:(t + 1) * P, :], in_=os[:])

            # ---- pipeline: prefix proj, then su-major attention with cbs ----
            qk_proj(wk_sb, xk_sb, bk_sb, KT, 0, 0)
            qk_proj(wq_sb, xq_sb, bq_sb, QT, 0, 0)
            qk_proj(wq_sb, xq_sb, bq_sb, QT, 0, 1)
            v_proj(0)
            v_proj(1)

            cbs_list = {
                (0, 0): {
                    0: [lambda: v_proj(2), lambda: v_proj(3)],
                    1: [lambda: v_proj(4), lambda: v_proj(5)],
                    2: [lambda: v_proj(6), lambda: v_proj(7),
                        lambda: qk_proj(wk_sb, xk_sb, bk_sb, KT, 0, 1)],
                    3: [lambda: v_proj(8), lambda: v_proj(9)],
                    4: [lambda: v_proj(10), lambda: v_proj(11)],
                    5: [lambda: v_proj(12), lambda: v_proj(13),
                        lambda: qk_proj(wk_sb, xk_sb, bk_sb, KT, 0, 2)],
                    6: [lambda: v_proj(14), lambda: v_proj(15)],
                    8: [lambda: qk_proj(wk_sb, xk_sb, bk_sb, KT, 0, 3)],
                    10: [lambda: qk_proj(wk_sb, xk_sb, bk_sb, KT, 1, 0)],
                    11: [lambda: qk_proj(wk_sb, xk_sb, bk_sb, KT, 1, 1)],
                    12: [lambda: qk_proj(wk_sb, xk_sb, bk_sb, KT, 1, 2)],
                    13: [lambda: qk_proj(wk_sb, xk_sb, bk_sb, KT, 1, 3)],
                },
                (0, 1): {
                    0: [lambda: qk_proj(wq_sb, xq_sb, bq_sb, QT, 1, 0)],
                    2: [lambda: qk_proj(wq_sb, xq_sb, bq_sb, QT, 1, 1)],
                    4: [lambda: qk_proj(wq_sb, xq_sb, bq_sb, QT, 0, 2)],
                    6: [lambda: qk_proj(wq_sb, xq_sb, bq_sb, QT, 0, 3)],
                    8: [lambda: qk_proj(wq_sb, xq_sb, bq_sb, QT, 1, 2)],
                    10: [lambda: qk_proj(wq_sb, xq_sb, bq_sb, QT, 1, 3)],
                },
                (1, 0): {
                    1: [lambda: transp(0, 0, 0)],
                    3: [lambda: transp(0, 0, 1)],
                    5: [lambda: transp(0, 1, 0)],
                    7: [lambda: transp(0, 1, 1)],
                    9: [lambda: outproj(0)],
                    10: [lambda: outproj(1)],
                    11: [lambda: outproj(2)],
                    12: [lambda: outproj(3)],
                    13: [lambda: outproj(4)],
                    14: [lambda: outproj(5)],
                },
                (1, 1): {
                    1: [lambda: outproj(6)],
                    3: [lambda: outproj(7)],
                },
                (1, 3): {
                    1: [lambda: transp(1, 0, 0)],
                    3: [lambda: transp(1, 0, 1)],
                },
            }

            for su in range(NSUP):
                for h in range(NH):
                    emit_jloop(su, h, cbs_list.get((su, h), {}))

            # tail: remaining transposes, remaining outproj
            transp(1, 1, 0)
            transp(1, 1, 1)
            for t in range(8, 16):
                outproj(t)

    nc.compile()
    return nc


def _prep_in_maps(query, key, value, mask, Wq, bq, Wk, bk, Wv, bv, Wo):
    ident = np.eye(P, dtype=np.float32)
    bfl = ml_dtypes.bfloat16
    in_maps = []
    xT_cache = {}
    for b in range(4):
        xT_cache[b] = (
            np.ascontiguousarray(query[b].T).astype(bfl),
            np.ascontiguousarray(key[b].T).astype(bfl),
            np.ascontiguousarray(value[b].T).astype(bfl),
        )
    for c in range(8):
        b = c // 2
        hh = c % 2
        ob = slice(hh * OB, (hh + 1) * OB)
        mbias = np.where(mask[b, 0, :] == 0, np.float32(NEG), np.float32(0.0))
        mbias = np.ascontiguousarray(mbias.reshape(JT, P).T)
        qT, kT, vT = xT_cache[b]
        in_maps.append({
            "xqT": qT,
            "xkT": kT,
            "xvT": vT,
            "wqT": np.ascontiguousarray(Wq[ob, :].T).astype(bfl),
            "wkT": np.ascontiguousarray(Wk[ob, :].T).astype(bfl),
            "wvT": np.ascontiguousarray(Wv[ob, :].T).astype(bfl),
            "woT": np.ascontiguousarray(Wo[:, ob].T),
            "bqr": np.ascontiguousarray(bq[ob].reshape(OB // P, P).T),
            "bkr": np.ascontiguousarray(bk[ob].reshape(OB // P, P).T),
            "bvb": np.ascontiguousarray(np.tile(bv[ob][None, :], (P, 1))),
            "mb": mbias,
            "ident": ident,
        })
    return in_maps


def kernel(query, key, value, mask, Wq, bq, Wk, bk, Wv, bv, Wo, bo):
    query = np.asarray(query, dtype=np.float32)
    key = np.asarray(key, dtype=np.float32)
    value = np.asarray(value, dtype=np.float32)
    mask = np.asarray(mask)
    Wq = np.asarray(Wq, dtype=np.float32)
    bq = np.asarray(bq, dtype=np.float32)
    Wk = np.asarray(Wk, dtype=np.float32)
    bk = np.asarray(bk, dtype=np.float32)
    Wv = np.asarray(Wv, dtype=np.float32)
    bv = np.asarray(bv, dtype=np.float32)
    Wo = np.asarray(Wo, dtype=np.float32)
    bo = np.asarray(bo, dtype=np.float32)

    if "nc" not in _CACHE:
        _CACHE["nc"] = _build()
    nc = _CACHE["nc"]

    B = query.shape[0]
    in_maps = _prep_in_maps(query, key, value, mask, Wq, bq, Wk, bk, Wv, bv, Wo)
    res = run_bass_kernel_spmd(nc, in_maps, core_ids=list(range(8)))

    out = np.empty((B, T, F), dtype=np.float32)
    for b in range(B):
        out[b] = res.results[2 * b]["out"] + res.results[2 * b + 1]["out"] + bo[None, :]
    return out


# revision 23
# speedup vs baseline: 1.6137x; 1.1685x over previous
"""Multi-head attention Trainium2 kernel (8 NeuronCores, SPMD), v2.

Problem: B=4, T=2048, n_feat=512, H=8 heads, d_k=64.
Sharding: core c -> batch b = c//2, head-half hh = c%2 (4 heads = 256 attn dims).

Per-core dataflow (v2):
  - Host passes x^T [F, T] and W^T [F, OB] in bf16, Wo^T [OB, F] in f32 ->
    no on-device transposes for the projection phase.
  - Q^T/K^T [o, t] via W^T-stationary matmuls; V [t, o] natural + ones column
    (bf16) for the softmax denominator.
  - Attention: scores transposed S^T[j, i] (f32r, N=512); exp on ACT into
    bf16 E^T tiles; PV flipped: lhsT = E^T chunk, rhs = V||1 (bf16, N=65)
    accumulating x[i, o] + Z[i] in PSUM -> normalization is partition-parallel
    (reciprocal + broadcast multiply on DVE only), inline at loop end.
  - x blocks transposed on PE (f32r identity), out = x @ Wo^T in natural
    [T, F] layout; host sums the two head-half partials per batch and adds bo.
"""
import sys

sys.path.insert(0, "/opt/trn_rl_repo")

import numpy as np
import ml_dtypes

import concourse.bass as bass
import concourse.tile as tile
from concourse import bacc, mybir
from concourse.bass import broadcast_tensor_aps
from concourse.bass_utils import run_bass_kernel_spmd

P = 128
T = 2048
F = 512            # n_feat (projection contraction dim)
OB = 256           # per-core attention dims (4 heads x 64)
NH = 4             # local heads
DK = 64
NT = T // P        # 16 row tiles
FO = F // P        # 4 feature tiles
NSUP = 2           # i-supers
ISUP = T // NSUP   # 1024
JT = NT            # 16 j tiles
NEG = -1.0e30
EPS = 1e-8
DW = DK + 1        # PV output width per i-tile (x + Z)
# bf16-Schraudolph fast-exp constants (DVE offload of some exp tiles):
# int16 bits = round(x * 128/ln2 + (16256 - C)); C calibrated, ~2% rms.
FE_C1 = 128.0 / np.log(2.0) / 8.0      # folds the 1/sqrt(d_k)=1/8 scale
FE_MASKMUL = 128.0 / np.log(2.0)
FE_MAGIC = 16256.0 - 5.25
FE_JTS = (3, 7, 11)                    # j-tiles computed on DVE instead of ACT

f32 = mybir.dt.float32
f32r = mybir.dt.float32r
bf16 = mybir.dt.bfloat16

_CACHE = {}


def _build():
    nc = bacc.Bacc("TRN2", target_bir_lowering=False, debug=False, num_devices=8)

    xqT = nc.dram_tensor("xqT", (F, T), bf16, kind="ExternalInput").ap()
    xkT = nc.dram_tensor("xkT", (F, T), bf16, kind="ExternalInput").ap()
    xvT = nc.dram_tensor("xvT", (F, T), bf16, kind="ExternalInput").ap()
    # wqk: [WqT | WkT] merged; cst: [bqr(2) | bkr(2) | mb(16) | mbc(16) | bvb(256)]
    wqk = nc.dram_tensor("wqk", (F, 2 * OB), bf16, kind="ExternalInput").ap()
    wvT = nc.dram_tensor("wvT", (F, OB), bf16, kind="ExternalInput").ap()
    woT = nc.dram_tensor("woT", (OB, F), f32r, kind="ExternalInput").ap()
    cst = nc.dram_tensor("cst", (P, 292), f32, kind="ExternalInput").ap()
    ident = nc.dram_tensor("ident", (P, P), f32r, kind="ExternalInput").ap()
    outD = nc.dram_tensor("out", (T, F), f32, kind="ExternalOutput").ap()

    with tile.TileContext(nc) as tc:
        with tc.tile_pool(name="const", bufs=1) as cpool, \
             tc.tile_pool(name="xin", bufs=1) as xpool, \
             tc.tile_pool(name="persist", bufs=1) as ppool, \
             tc.tile_pool(name="et", bufs=3) as epool, \
             tc.tile_pool(name="norm", bufs=2) as npool, \
             tc.tile_pool(name="ost", bufs=8) as opool, \
             tc.tile_pool(name="ps", bufs=2, space="PSUM") as ps, \
             tc.tile_pool(name="psacc", bufs=1, space="PSUM") as psa:

            # ---- warm-up: hoist the activation-table load off the fill path
            warm = cpool.tile([1, 2], f32, tag="warm")
            nc.vector.memset(warm[0:1, 0:1], 0.0)
            nc.scalar.activation(
                warm[0:1, 1:2], warm[0:1, 0:1], mybir.ActivationFunctionType.Exp
            )

            # ---- constant / weight loads (SP queue), fill-critical first ----
            cst_sb = cpool.tile([P, 292], f32, tag="cst")
            nc.sync.dma_start(out=cst_sb[:], in_=cst[:])
            bq_sb = cst_sb[:, 0:2]
            bk_sb = cst_sb[:, 2:4]
            mb_sb = cst_sb[:, 4:20]
            bv_sb = cst_sb[:, 36:292]

            # wqk layout: [wq_o0 | wk_o0 | wq_o1 | wk_o1], 128 cols each --
            # the o-tile-0 half loads first (fill-critical), o-tile 1 later.
            wqk_sb = cpool.tile([P, FO, 2 * OB], bf16, tag="wqk")
            wqkr = wqk.rearrange("(fo p) o -> p fo o", p=P)
            nc.sync.dma_start(out=wqk_sb[:, :, 0:OB], in_=wqkr[:, :, 0:OB])

            xq_sb = xpool.tile([P, FO, T], bf16, tag="xq")
            xk_sb = xpool.tile([P, FO, T], bf16, tag="xk")
            xv_sb = xpool.tile([P, FO, T], bf16, tag="xv")
            xkr = xkT.rearrange("(fo p) t -> p fo t", p=P)
            xqr = xqT.rearrange("(fo p) t -> p fo t", p=P)
            xvr = xvT.rearrange("(fo p) t -> p fo t", p=P)

            def xdma(sb, dr, lo, hi):
                nc.sync.dma_start(out=sb[:, :, lo:hi], in_=dr[:, :, lo:hi])

            xdma(xk_sb, xkr, 0, 256)
            xdma(xq_sb, xqr, 0, 512)
            xdma(xq_sb, xqr, 512, 1024)
            wv_sb = cpool.tile([P, FO, OB], bf16, tag="wv")
            nc.sync.dma_start(out=wv_sb[:], in_=wvT.rearrange("(fo p) o -> p fo o", p=P))
            xdma(xv_sb, xvr, 0, 512)
            xdma(xk_sb, xkr, 256, 512)
            xdma(xk_sb, xkr, 512, 1024)
            xdma(xv_sb, xvr, 512, 1024)
            xdma(xk_sb, xkr, 1024, 1536)
            xdma(xv_sb, xvr, 1024, 1536)
            xdma(xk_sb, xkr, 1536, 2048)
            xdma(xv_sb, xvr, 1536, 2048)
            xdma(xq_sb, xqr, 1024, 1536)
            xdma(xq_sb, xqr, 1536, 2048)
            nc.sync.dma_start(out=wqk_sb[:, :, OB:2 * OB], in_=wqkr[:, :, OB:2 * OB])

            id_sb = cpool.tile([P, P], f32r, tag="ident")
            nc.sync.dma_start(out=id_sb[:], in_=ident[:])
            wo_sb = cpool.tile([P, 2, F], f32r, tag="wo")
            nc.sync.dma_start(out=wo_sb[:], in_=woT.rearrange("(pc p) f -> p pc f", p=P))

            # ---- persistent activations ----
            QT = ppool.tile([P, OB // P, T], f32r, tag="QT")
            KT = ppool.tile([P, OB // P, T], f32r, tag="KT")
            V2 = ppool.tile([P, NT, NH, DW], bf16, tag="V2")
            xnorm = ppool.tile([P, NT, OB], f32r, tag="xnorm")
            xT2 = ppool.tile([P, 2, T], f32r, tag="xT2")
            nc.vector.memset(V2[:, :, :, DK:DW], 1.0)

            # ---- projection pieces (interleaved into attention via cbs) ----
            def qk_proj(w_sb, x_sb, bias_sb, dst, otile, lo, hi):
                pp = ps.tile([P, F], f32, tag="pp")
                for fo in range(FO):
                    nc.tensor.matmul(
                        pp[:, 0:hi - lo],
                        w_sb[:, fo, otile * P:(otile + 1) * P],
                        x_sb[:, fo, lo:hi],
                        start=(fo == 0),
                        stop=(fo == FO - 1),
                    )
                nc.vector.tensor_scalar_add(
                    dst[:, otile, lo:hi],
                    pp[:, 0:hi - lo],
                    bias_sb[:, otile:otile + 1],
                )

            def v_proj(t):
                pp = ps.tile([P, F], f32, tag="pp")
                for fo in range(FO):
                    nc.tensor.matmul(
                        pp[:, :OB],
                        xv_sb[:, fo, t * P:(t + 1) * P],
                        wv_sb[:, fo, :],
                        start=(fo == 0),
                        stop=(fo == FO - 1),
                    )
                nc.vector.tensor_add(
                    V2[:, t, :, 0:DK],
                    pp[:, :OB].rearrange("p (h d) -> p h d", h=NH),
                    bv_sb[:].rearrange("p (h d) -> p h d", h=NH),
                )

            # ---- normalization (DVE only; one PSUM read decouples WAR) ----
            def emit_norm(su, h, acc):
                nsb = npool.tile([P, 2, 4, DW], f32, tag="nsb")
                nc.vector.tensor_copy(
                    nsb[:],
                    acc[:, :, 0:4 * DW].rearrange("p g (r c) -> p g r c", c=DW),
                )
                zz = npool.tile([P, 8], f32, tag="zz")
                nc.vector.tensor_scalar_add(
                    zz[:], nsb[:, :, :, DK].rearrange("p g r -> p (g r)"), EPS
                )
                rr = npool.tile([P, 8], f32, tag="rr")
                nc.vector.reciprocal(rr[:], zz[:])
                for g in (0, 1):
                    dst = xnorm[:, su * 8 + g * 4:su * 8 + (g + 1) * 4,
                                h * DK:(h + 1) * DK]
                    r_ap = rr[:, g * 4:(g + 1) * 4].rearrange("p a -> p a ()")
                    r_b, a_b = broadcast_tensor_aps(r_ap, nsb[:, g, :, 0:DK])
                    nc.vector.tensor_mul(dst, a_b, r_b)

            # ---- attention j-loop ----
            def emit_jloop(su, h, cbs, cbs_pre=None):
                qoff = (h % 2) * DK
                qpo = h // 2
                isl = su * ISUP
                acc = psa.tile([P, 2, F], f32, tag="acc")

                def scores_mm(dst_ap, jt, cc):
                    nc.tensor.matmul(
                        dst_ap,
                        KT[qoff:qoff + DK, qpo, jt * P:(jt + 1) * P],
                        QT[qoff:qoff + DK, qpo, isl + cc * F:isl + (cc + 1) * F],
                        start=True,
                        stop=True,
                    )

                def scores(jt):
                    if jt in FE_JTS:
                        # FE tiles take the two "pp" slots: keeps the "big"
                        # double-buffer free so ACT never waits on the
                        # et->PV->scores chain around a DVE-exp tile.
                        a = ps.tile([P, F], f32, tag="pp")
                        b = ps.tile([P, F], f32, tag="pp")
                        scores_mm(a[:], jt, 0)
                        scores_mm(b[:], jt, 1)
                        return (a, b)
                    st = ps.tile([P, ISUP], f32, tag="big")
                    for cc in range(ISUP // F):
                        scores_mm(st[:, cc * F:(cc + 1) * F], jt, cc)
                    return st

                st_prev = scores(0)
                for jt in range(JT):
                    et = epool.tile([P, ISUP], bf16, tag="et")
                    if jt in FE_JTS:
                        # bf16-Schraudolph exp on DVE (ACT is the bottleneck):
                        # int16 bits = (st * C1) + mbc, saturating convert.
                        mcol = cst_sb[:, 20 + jt:21 + jt]
                        for cc, stp in enumerate(st_prev):
                            i1, _ = broadcast_tensor_aps(mcol, stp[:])
                            nc.vector.scalar_tensor_tensor(
                                et[:, cc * F:(cc + 1) * F].bitcast(mybir.dt.int16),
                                stp[:],
                                FE_C1,
                                i1,
                                mybir.AluOpType.mult,
                                mybir.AluOpType.add,
                            )
                    else:
                        nc.scalar.activation(
                            et[:],
                            st_prev[:],
                            mybir.ActivationFunctionType.Exp,
                            bias=mb_sb[:, jt:jt + 1],
                            scale=0.125,
                        )
                    if jt + 1 < JT:
                        st_prev = scores(jt + 1)
                    if cbs_pre:
                        for cb in cbs_pre.get(jt, ()):
                            cb()
                    for it in range(8):
                        # PSUM start zeroes the whole 2KB bank: only the first
                        # packed region per bank starts, only the last stops.
                        nc.tensor.matmul(
                            acc[:, it // 4, (it % 4) * DW:(it % 4) * DW + DW],
                            et[:, it * P:(it + 1) * P],
                            V2[:, jt, h, :],
                            start=(jt == 0 and it % 4 == 0),
                            stop=(jt == JT - 1 and it % 4 == 3),
                        )
                    for cb in cbs.get(jt, ()):
                        cb()
                emit_norm(su, h, acc)

            # ---- x transpose + output projection pieces ----
            def transp(su, pc, half):
                tp = ps.tile([P, F], f32r, tag="pp")
                for k in range(4):
                    t = su * 8 + half * 4 + k
                    nc.tensor.transpose(
                        tp[:, k * P:(k + 1) * P],
                        xnorm[:, t, pc * P:(pc + 1) * P],
                        id_sb[:],
                    )
                nc.vector.tensor_copy(
                    xT2[:, pc, (su * 8 + half * 4) * P:(su * 8 + half * 4) * P + F],
                    tp[:],
                )

            def outproj(t, eng="dve"):
                pp = ps.tile([P, F], f32, tag="pp")
                for pc in range(2):
                    nc.tensor.matmul(
                        pp[:],
                        xT2[:, pc, t * P:(t + 1) * P],
                        wo_sb[:, pc, :],
                        start=(pc == 0),
                        stop=(pc == 1),
                    )
                os = opool.tile([P, F], f32, tag="os")
                if eng == "act":
                    nc.scalar.copy(os[:], pp[:])
                else:
                    nc.vector.tensor_copy(os[:], pp[:])
                nc.sync.dma_start(out=outD[t * P:(t + 1) * P, :], in_=os[:])

            # ---- pipeline: prefix proj, then su-major attention with cbs ----
            def K_(ot, lo, hi):
                return lambda: qk_proj(wk_sb, xk_sb, bk_sb, KT, ot, lo, hi)

            def Q_(ot, lo, hi):
                return lambda: qk_proj(wq_sb, xq_sb, bq_sb, QT, ot, lo, hi)

            qk_proj(wk_sb, xk_sb, bk_sb, KT, 0, 0, 256)
            qk_proj(wq_sb, xq_sb, bq_sb, QT, 0, 0, 512)
            qk_proj(wq_sb, xq_sb, bq_sb, QT, 0, 512, 1024)

            # piece slots avoid jt in {2,3,6,7,10,11}: FE scores (jt in 3,7,11)
            # occupy the two "pp" psum slots during iterations jt-1 and jt.
            cbs_pre_list = {
                (0, 0): {0: [lambda: v_proj(0), lambda: v_proj(1)]},
            }
            cbs_list = {
                (0, 0): {
                    0: [lambda: v_proj(2), lambda: v_proj(3), K_(0, 256, 512)],
                    1: [lambda: v_proj(4), lambda: v_proj(5), K_(0, 512, 1024)],
                    4: [lambda: v_proj(6), lambda: v_proj(7)],
                    5: [lambda: v_proj(8), lambda: v_proj(9), K_(0, 1024, 1536)],
                    8: [lambda: v_proj(10), lambda: v_proj(11), K_(0, 1536, 2048)],
                    9: [lambda: v_proj(12), lambda: v_proj(13)],
                    12: [lambda: v_proj(14), lambda: v_proj(15)],
                },
                (0, 1): {
                    0: [K_(1, 0, 512)],
                    1: [K_(1, 512, 1024)],
                    4: [K_(1, 1024, 1536)],
                    5: [K_(1, 1536, 2048)],
                    8: [Q_(1, 0, 512)],
                    9: [Q_(1, 512, 1024)],
                    12: [Q_(0, 1024, 1536)],
                    13: [Q_(0, 1536, 2048)],
                },
                (0, 2): {
                    0: [Q_(1, 1024, 1536)],
                    1: [Q_(1, 1536, 2048)],
                },
                (1, 0): {
                    0: [lambda: transp(0, 0, 0)],
                    1: [lambda: transp(0, 0, 1)],
                    4: [lambda: transp(0, 1, 0)],
                    5: [lambda: transp(0, 1, 1)],
                    8: [lambda: outproj(0)],
                    9: [lambda: outproj(1)],
                    12: [lambda: outproj(2)],
                    13: [lambda: outproj(3)],
                    14: [lambda: outproj(4)],
                    15: [lambda: outproj(5)],
                },
                (1, 1): {
                    0: [lambda: outproj(6)],
                    1: [lambda: outproj(7)],
                },
                (1, 3): {
                    0: [lambda: transp(1, 0, 0)],
                    1: [lambda: transp(1, 0, 1)],
                },
            }

            for su in range(NSUP):
                for h in range(NH):
                    emit_jloop(su, h, cbs_list.get((su, h), {}),
                               cbs_pre_list.get((su, h)))

            # tail: pc1/su1 transposes + su1 outproj, copies alternate ACT/DVE
            transp(1, 1, 0)
            for i, t in enumerate(range(8, 12)):
                outproj(t, "act" if i % 2 == 0 else "dve")
            transp(1, 1, 1)
            for i, t in enumerate(range(12, 16)):
                outproj(t, "act" if i % 2 == 0 else "dve")

    nc.compile()
    return nc


def _prep_in_maps(query, key, value, mask, Wq, bq, Wk, bk, Wv, bv, Wo):
    ident = np.eye(P, dtype=np.float32)
    bfl = ml_dtypes.bfloat16
    in_maps = []
    xT_cache = {}
    for b in range(4):
        xT_cache[b] = (
            np.ascontiguousarray(query[b].T).astype(bfl),
            np.ascontiguousarray(key[b].T).astype(bfl),
            np.ascontiguousarray(value[b].T).astype(bfl),
        )
    for c in range(8):
        b = c // 2
        hh = c % 2
        ob = slice(hh * OB, (hh + 1) * OB)
        mbias = np.where(mask[b, 0, :] == 0, np.float32(NEG), np.float32(0.0))
        mbias = np.ascontiguousarray(mbias.reshape(JT, P).T)
        qT, kT, vT = xT_cache[b]
        # cst: [bqr(2) | bkr(2) | mb(16) | mbc(16, reserved) | bvb(256)]
        cst = np.zeros((P, 292), np.float32)
        cst[:, 0:2] = bq[ob].reshape(OB // P, P).T
        cst[:, 2:4] = bk[ob].reshape(OB // P, P).T
        cst[:, 4:20] = mbias
        cst[:, 20:36] = mbias * np.float32(FE_MASKMUL) + np.float32(FE_MAGIC)
        cst[:, 36:292] = bv[ob][None, :]
        in_maps.append({
            "xqT": qT,
            "xkT": kT,
            "xvT": vT,
            "wqk": np.ascontiguousarray(
                np.concatenate([Wq[ob, :].T, Wk[ob, :].T], axis=1)
            ).astype(bfl),
            "wvT": np.ascontiguousarray(Wv[ob, :].T).astype(bfl),
            "woT": np.ascontiguousarray(Wo[:, ob].T),
            "cst": cst,
            "ident": ident,
        })
    return in_maps


def kernel(query, key, value, mask, Wq, bq, Wk, bk, Wv, bv, Wo, bo):
    query = np.asarray(query, dtype=np.float32)
    key = np.asarray(key, dtype=np.float32)
    value = np.asarray(value, dtype=np.float32)
    mask = np.asarray(mask)
    Wq = np.asarray(Wq, dtype=np.float32)
    bq = np.asarray(bq, dtype=np.float32)
    Wk = np.asarray(Wk, dtype=np.float32)
    bk = np.asarray(bk, dtype=np.float32)
    Wv = np.asarray(Wv, dtype=np.float32)
    bv = np.asarray(bv, dtype=np.float32)
    Wo = np.asarray(Wo, dtype=np.float32)
    bo = np.asarray(bo, dtype=np.float32)

    if "nc" not in _CACHE:
        _CACHE["nc"] = _build()
    nc = _CACHE["nc"]

    B = query.shape[0]
    in_maps = _prep_in_maps(query, key, value, mask, Wq, bq, Wk, bk, Wv, bv, Wo)
    res = run_bass_kernel_spmd(nc, in_maps, core_ids=list(range(8)))

    out = np.empty((B, T, F), dtype=np.float32)
    for b in range(B):
        out[b] = res.results[2 * b]["out"] + res.results[2 * b + 1]["out"] + bo[None, :]
    return out
